# revision 6
# baseline (speedup 1.0000x reference)
"""Trainium2 Bass kernel for nn_CombinedModel (GCN message passing + MLPs).

Self-contained: takes FULL inputs (as produced by setup_inputs), shards across
8 NeuronCores internally, runs one SPMD Bass program per launch, returns the
FULL [4096, 1] output.

Design:
  - Nodes/graphs sharded across 8 cores at graph-aligned boundaries (dst
    sharding); per-core segment reductions over incoming edges.
  - GCN conv refactored as out = dinv * segsum(edges, dinv*x) @ W + b, so all
    edge aggregation happens in 64 features. Segment-sum is done per 128-dst
    block with selection-mask matmuls accumulated in PSUM (no scatter).
  - Edge source rows are fetched with the GPSIMD dma_gather custom op from
    f32 tables; int16 index range is handled by splitting tables into 32768-row
    windows and grouping each block's edges by window (host-side sort).
  - conv1 -> conv2 hand-off of the scaled feature table goes through an
    AllGather collective.
  - Pooling is another mask matmul over batch ids; the small MLPs run as
    plain PE matmuls with feature-on-partition layouts (no transposes
    anywhere in the kernel).
All heavy float math runs on device; the host only computes integer/layout
metadata (sorting, binning, index packing, degree counts).
"""
import math
import time
import contextlib
import numpy as np

import jax
from jax.sharding import Mesh, PartitionSpec, NamedSharding
from jax.experimental.shard_map import shard_map

import concourse.bass as bass
import concourse.bacc as bacc
import concourse.tile as tile
from concourse import mybir
from concourse.bass2jax import (
    _bass_exec_p,
    install_neuronx_cc_hook,
    partition_id_tensor,
)
from concourse.tile_rust import add_dep_helper

# ---------------- problem constants (hardcoded per the task spec) -----------
N = 131072
B = 4096
NCORES = 8
P = 128
H = 64
D_EMB = 768
WIN = 32768          # int16-addressable table window (rows)
F32 = mybir.dt.float32
F16 = mybir.dt.float16
I16 = mybir.dt.int16

GATHER_DEPTH = 6     # in-flight dma_gather instructions
NQ = 4               # SWDGE queues
SEC = 4              # blocks per gather section


# ---------------- host-side preprocessing ----------------------------------
def _preprocess(edge_index, batch):
    src = np.asarray(edge_index[0], dtype=np.int64)
    dst = np.asarray(edge_index[1], dtype=np.int64)
    batch = np.asarray(batch, dtype=np.int64)

    loops = np.arange(N, dtype=np.int64)
    src_all = np.concatenate([src, loops])
    dst_all = np.concatenate([dst, loops])
    deg = np.bincount(dst_all, minlength=N).astype(np.int64)

    gstart = np.searchsorted(batch, np.arange(B + 1))
    tgt = (np.arange(NCORES + 1) * N) // NCORES
    bnd_g = np.clip(np.searchsorted(gstart, tgt), 0, B)
    for c in range(1, NCORES):
        g = bnd_g[c]
        if g > 0 and abs(int(gstart[g - 1]) - tgt[c]) < abs(int(gstart[g]) - tgt[c]):
            bnd_g[c] = g - 1
    bnd_g[0] = 0
    bnd_g[NCORES] = B
    node_bnd = gstart[bnd_g]

    NBLK = 0
    cores = []
    for c in range(NCORES):
        n0, n1 = int(node_bnd[c]), int(node_bnd[c + 1])
        cores.append(dict(n0=n0, n1=n1, g0=int(bnd_g[c]), g1=int(bnd_g[c + 1])))
        NBLK = max(NBLK, (n1 - n0 + P - 1) // P)

    for co in cores:
        n0, n1 = co["n0"], co["n1"]
        Vc = n1 - n0
        mask = (dst_all >= n0) & (dst_all < n1)
        co["e_src"] = src_all[mask]
        e_dst = dst_all[mask] - n0
        ldeg = deg[n0:n1]
        order = np.argsort(-ldeg, kind="stable")
        blk_of = np.empty(Vc, dtype=np.int64)
        blk_of[order] = np.arange(Vc, dtype=np.int64) % NBLK
        slot_of = np.empty(Vc, dtype=np.int64)
        for b in range(NBLK):
            sel = order[blk_of[order] == b]
            slot_of[sel] = np.arange(len(sel))
        perm = -np.ones(NBLK * P, dtype=np.int64)
        perm[blk_of * P + slot_of] = np.arange(Vc)
        co.update(Vc=Vc, Gc=co["g1"] - co["g0"], blk_of=blk_of, slot_of=slot_of,
                  perm=perm, e_blk=blk_of[e_dst], e_slot=slot_of[e_dst])
    return dict(cores=cores, deg=deg, NBLK=NBLK)


def _build_chunks(meta, key_fn, nwin):
    """Per core: per (block, window) chunked edge lists, padded to x128.
    key_fn maps global src node id -> table position. Returns per-core dicts +
    NCBQ (chunks per window, maxed over cores & blocks)."""
    NBLK = meta["NBLK"]
    pc = []
    counts = np.zeros((len(meta["cores"]), NBLK, nwin), dtype=np.int64)
    for ic, co in enumerate(meta["cores"]):
        key = key_fn(co["e_src"])
        w = key // WIN
        order = np.lexsort((key, w, co["e_blk"]))
        s_key, s_w, s_blk = key[order], w[order], co["e_blk"][order]
        s_slot = co["e_slot"][order]
        np.add.at(counts[ic], (s_blk, s_w), 1)
        pc.append((s_key, s_w, s_blk, s_slot))
    NCBQ = ((counts.max(axis=(0, 1)) + P - 1) // P).astype(np.int64)
    NCB = int(NCBQ.sum())
    out = []
    for ic, (s_key, s_w, s_blk, s_slot) in enumerate(pc):
        ckey = np.zeros((NBLK, NCB, P), dtype=np.int64)
        cslot = np.full((NBLK, NCB, P), 255, dtype=np.int64)
        blk_lo = np.searchsorted(s_blk, np.arange(NBLK + 1))
        for b in range(NBLK):
            bk = s_key[blk_lo[b]:blk_lo[b + 1]]
            bw = s_w[blk_lo[b]:blk_lo[b + 1]]
            bs = s_slot[blk_lo[b]:blk_lo[b + 1]]
            ci = 0
            for w in range(nwin):
                lo, hi = np.searchsorted(bw, [w, w + 1])
                k = hi - lo
                nch = int(NCBQ[w])
                assert k <= nch * P, f"window overflow b={b} w={w} k={k}"
                ckey[b, ci:ci + nch] = w * WIN
                for j in range((k + P - 1) // P):
                    a, z = lo + j * P, lo + min((j + 1) * P, k)
                    ckey[b, ci + j, : z - a] = bk[a:z]
                    cslot[b, ci + j, : z - a] = bs[a:z]
                ci += nch
        out.append(dict(ckey=ckey, cslot=cslot))
    return out, NCBQ, NCB


def _pack_idx(ckey, NCBQ):
    """[NBLK, NCB, 128] table positions -> int16 idx array [128, NBLK*NCB*8]
    in dma_gather firmware layout (i%16 wrap + 8x replication), window-local."""
    NBLK, NCB, _ = ckey.shape
    nwin = len(NCBQ)
    out = np.zeros((P, NBLK * NCB * 8), dtype=np.int16)
    col = 0
    for b in range(NBLK):
        ci = 0
        for w in range(nwin):
            nch = int(NCBQ[w])
            flat = (ckey[b, ci:ci + nch].reshape(-1) - w * WIN).astype(np.int64)
            nidx = nch * P
            arr = np.zeros((16, nidx // 16), dtype=np.int16)
            arr[np.arange(nidx) % 16, np.arange(nidx) // 16] = flat.astype(np.int16)
            blockcols = nidx // 16
            for grp in range(8):
                out[grp * 16:(grp + 1) * 16, col:col + blockcols] = arr
            col += blockcols
            ci += nch
    assert col == NBLK * NCB * 8
    return out


def _bc(ap, extra):
    """Append broadcast dims ([0, n] entries) to an AP."""
    return bass.AP(ap.tensor, ap.offset, list(ap.ap) + [[0, n] for n in extra])


def _mid_bc(ap2d, ng):
    """[P, F] AP -> [P, ng(bcast), F]."""
    a = ap2d.ap
    return bass.AP(ap2d.tensor, ap2d.offset, [list(a[0]), [0, ng], list(a[1])])


# ---------------- kernel builder -------------------------------------------
def build_kernel(cfg):
    NBLK = cfg["NBLK"]
    V_pad = NBLK * P
    G_pad = cfg["G_pad"]
    NCBQ1, NCB1 = cfg["NCBQ1"], cfg["NCB1"]
    NCBQ2, NCB2 = cfg["NCBQ2"], cfg["NCB2"]
    NW1, NW2 = len(NCBQ1), len(NCBQ2)
    SLAB = NCORES * V_pad

    nc = bacc.Bacc("TRN2", target_bir_lowering=False, num_devices=NCORES,
                   num_swdge_queues=NQ)

    def din(name, shape, dt=F32):
        return nc.dram_tensor(name, shape, dt, kind="ExternalInput")

    x_pm = din("x_pm", [P, (N // P) * H], F16)
    deg_pm = din("deg_pm", [P, N // P])
    deg_perm = din("deg_perm", [P, NBLK])
    batchl = din("batchl", [P, NBLK])
    cslot1 = din("cslot1", [P, NBLK * NCB1])
    cslot2 = din("cslot2", [P, NBLK * NCB2])
    idx1 = din("idx1", [P, NBLK * NCB1 * 8], I16)
    idx2 = din("idx2", [P, NBLK * NCB2 * 8], I16)
    smilesT = din("smilesT", [D_EMB, G_pad])
    cntg = din("cntg", [P, G_pad])
    iota128 = din("iota128", [P, P])
    giota = din("giota", [P, G_pad])
    emb_W1 = din("emb_W1", [D_EMB, 1024])
    emb_b1 = din("emb_b1", [1024, 1])
    emb_W2 = din("emb_W2", [1024, H])
    emb_b2 = din("emb_b2", [H, 1])
    conv1_W = din("conv1_W", [H, H])
    conv1_b = din("conv1_b", [P, H])
    conv2_W = din("conv2_W", [H, 2 * H])
    conv2_b = din("conv2_b", [P, 2 * H])
    gcn_fc_W = din("gcn_fc_W", [2 * H, H])
    gcn_fc_b = din("gcn_fc_b", [H, 1])
    fc1_W = din("fc1_W", [2 * H, H])
    fc1_b = din("fc1_b", [H, 1])
    fcf_W = din("fcf_W", [H, 1])
    fcf_b = din("fcf_b", [1, 1])

    out_d = nc.dram_tensor("out", [1, G_pad], F32, kind="ExternalOutput")

    NT = N // P  # 1024 p-major tiles

    with contextlib.ExitStack() as st:
        sem_g = st.enter_context(nc.semaphore("sem_gather"))
        tc = st.enter_context(tile.TileContext(nc))
        consts = st.enter_context(tc.tile_pool(name="consts", bufs=1))
        dram = st.enter_context(tc.tile_pool(name="dram", bufs=1, space="DRAM"))
        work = st.enter_context(tc.tile_pool(name="work", bufs=2))
        gpool = st.enter_context(tc.tile_pool(name="gpool", bufs=2 * SEC))
        mpool = st.enter_context(tc.tile_pool(name="mpool", bufs=3))
        psum = st.enter_context(tc.tile_pool(name="psum", bufs=2, space="PSUM"))
        psum1 = st.enter_context(tc.tile_pool(name="psum1", bufs=1, space="PSUM"))

        # ---- constants / small tensors in SBUF ----
        def load_const(name, src, shape, dt=F32):
            t = consts.tile(shape, dt, tag=name)
            nc.sync.dma_start(t[:], src[:])
            return t

        W1_t = load_const("W1", conv1_W, [H, H])
        b1_t = load_const("b1", conv1_b, [P, H])
        W2_t = load_const("W2", conv2_W, [H, 2 * H])
        b2_t = load_const("b2", conv2_b, [P, 2 * H])
        gfcW_t = load_const("gfcW", gcn_fc_W, [2 * H, H])
        gfcb_t = load_const("gfcb", gcn_fc_b, [H, 1])
        fc1Wa_t = consts.tile([H, H], F32, tag="fc1Wa")
        nc.sync.dma_start(fc1Wa_t[:], fc1_W[:H, :])
        fc1Wb_t = consts.tile([H, H], F32, tag="fc1Wb")
        nc.sync.dma_start(fc1Wb_t[:], fc1_W[H:, :])
        fc1b_t = load_const("fc1b", fc1_b, [H, 1])
        fcfW_t = load_const("fcfW", fcf_W, [H, 1])
        fcfb_t = load_const("fcfb", fcf_b, [1, 1])
        iota_t = load_const("iota", iota128, [P, P])
        giota_t = load_const("giota", giota, [P, G_pad])
        cnt_t = load_const("cnt", cntg, [P, G_pad])
        degp_t = load_const("degp", deg_perm, [P, NBLK])
        batchl_t = load_const("batchl", batchl, [P, NBLK])
        embW1_t = []
        for k in range(D_EMB // P):
            t = consts.tile([P, 1024], F32, tag=f"embW1_{k}")
            nc.sync.dma_start(t[:], emb_W1[k * P:(k + 1) * P, :])
            embW1_t.append(t)
        embW2_t = []
        for m in range(1024 // P):
            t = consts.tile([P, H], F32, tag=f"embW2_{m}")
            nc.sync.dma_start(t[:], emb_W2[m * P:(m + 1) * P, :])
            embW2_t.append(t)
        embb1_t = consts.tile([P, 1024 // P], F32, tag="embb1")
        nc.sync.dma_start(
            embb1_t[:], emb_b1[:].rearrange("(m p) o -> p (m o)", p=P))
        embb2_t = load_const("embb2", emb_b2, [H, 1])

        # dinv_perm = 1/sqrt(deg_perm)
        dinvp_t = consts.tile([P, NBLK], F32, tag="dinvp")
        nc.vector.reciprocal(dinvp_t[:], degp_t[:])
        nc.scalar.activation(dinvp_t[:], dinvp_t[:],
                             mybir.ActivationFunctionType.Sqrt)

        # cntinv = 1/max(cnt,1)
        cntinv_t = consts.tile([P, G_pad], F32, tag="cntinv")
        nc.vector.tensor_scalar_max(cntinv_t[:], cnt_t[:], 1.0)
        nc.vector.reciprocal(cntinv_t[:], cntinv_t[:])

        # ---- conv1 gather table: xs1 (p-major) = dinv * x, f32 in DRAM ----
        xs1 = dram.tile([P, NT * H], F32, tag="xs1")
        dinv_nat = consts.tile([P, NT], F32, tag="dinvnat")
        degn_t = consts.tile([P, NT], F32, tag="degn")
        nc.sync.dma_start(degn_t[:], deg_pm[:])
        nc.vector.reciprocal(dinv_nat[:], degn_t[:])
        nc.scalar.activation(dinv_nat[:], dinv_nat[:],
                             mybir.ActivationFunctionType.Sqrt)
        TB = 16  # tiles per table-build step
        for t0 in range(0, NT, TB):
            xc = work.tile([P, TB * H], F16, tag="xchunk")
            nc.sync.dma_start(xc[:], x_pm[:, t0 * H:(t0 + TB) * H])
            xo = work.tile([P, TB * H], F32, tag="xschunk")
            nc.vector.tensor_tensor(
                out=xo[:].rearrange("p (t d) -> p t d", d=H),
                in0=xc[:].rearrange("p (t d) -> p t d", d=H),
                in1=_bc(dinv_nat[:, t0:t0 + TB], [H]),
                op=mybir.AluOpType.mult)
            nc.sync.dma_start(xs1[:, t0 * H:(t0 + TB) * H], xo[:])
        xs1_rows = xs1[:].rearrange("p (t d) -> (p t) d", d=H)  # [N, 64] view

        # ---- collective buffers ----
        cc_in = dram.tile([V_pad, H], F32, tag="cc_in")
        cc_out = dram.tile([SLAB, H], F32, tag="cc_out", addr_space="Shared")

        gcount = [0]
        prev_crit = [None]

        def gather_section(blks, idx_t, sec_base_col, NCBQ, table_rows, win_rows):
            """Issue gathers for blocks in `blks`; returns dict (b, w) -> tile."""
            tiles = {}
            for b in blks:
                for w in range(len(NCBQ)):
                    nch = int(NCBQ[w])
                    g = gpool.tile([P, nch * H], F32, tag=f"g{w}")
                    tiles[(b, w)] = g
            first_inst = [None]
            with tc.tile_critical():
                for bi, b in enumerate(blks):
                    colb = sec_base_col[bi]
                    for w in range(len(NCBQ)):
                        nch = int(NCBQ[w])
                        nidx = nch * P
                        i = gcount[0]
                        if i >= GATHER_DEPTH:
                            inst = nc.gpsimd.wait_ge(sem_g, 16 * (i - GATHER_DEPTH + 1))
                            if first_inst[0] is None:
                                first_inst[0] = inst.ins
                        wlo = w * WIN
                        wr = min(WIN, win_rows - wlo)
                        inst = nc.gpsimd.dma_gather(
                            out_ap=tiles[(b, w)][:].rearrange(
                                "p (j d) -> p j d", d=H),
                            in_ap=table_rows[wlo:wlo + wr],
                            idxs_ap=idx_t[:, colb:colb + nidx // 16],
                            num_idxs=nidx, num_idxs_reg=nidx, elem_size=H,
                            queue_num=i % NQ)
                        inst.then_inc(sem_g, 16)
                        if first_inst[0] is None:
                            first_inst[0] = inst.ins
                        gcount[0] += 1
                        colb += nidx // 16
                last = nc.gpsimd.wait_ge(sem_g, 16 * gcount[0]).ins
            if prev_crit[0] is not None:
                add_dep_helper(prev_crit[0], first_inst[0], sync=False,
                               reason="gather section order")
            prev_crit[0] = last
            return tiles

        def conv(idx_d, cslot_d, NCBQ, NCB, table_rows, win_rows, Wl_t, bl_t,
                 fout, is_conv1):
            """Emit one conv pass. Per-block callback handles the tail."""
            NW = len(NCBQ)
            for s0 in range(0, NBLK, SEC):
                blks = list(range(s0, min(s0 + SEC, NBLK)))
                # section idx + cslot loads
                idx_t = mpool.tile([P, SEC * NCB * 8], I16, tag="idxsec")
                nc.sync.dma_start(
                    idx_t[:, :len(blks) * NCB * 8],
                    idx_d[:, s0 * NCB * 8:(s0 + len(blks)) * NCB * 8])
                cs_t = mpool.tile([P, SEC * NCB], F32, tag="cssec")
                nc.sync.dma_start(
                    cs_t[:, :len(blks) * NCB],
                    cslot_d[:, s0 * NCB:(s0 + len(blks)) * NCB])
                base_cols = [bi * NCB * 8 for bi in range(len(blks))]
                tiles = gather_section(blks, idx_t, base_cols, NCBQ,
                                       table_rows, win_rows)
                for bi, b in enumerate(blks):
                    aggp = psum.tile([H, P], F32, tag="agg")
                    ci = 0
                    for w in range(NW):
                        nch = int(NCBQ[w])
                        g = tiles[(b, w)]
                        for j in range(nch):
                            if ci % 4 == 0:
                                ng = min(4, NCB - ci)
                                mk = mpool.tile([P, 4 * P], F32, tag="mask")
                                nc.vector.tensor_tensor(
                                    out=mk[:, :ng * P].rearrange(
                                        "p (c q) -> p c q", q=P),
                                    in0=_bc(cs_t[:, bi * NCB + ci:
                                                 bi * NCB + ci + ng], [P]),
                                    in1=_mid_bc(iota_t[:], ng),
                                    op=mybir.AluOpType.is_equal)
                            nc.tensor.matmul(
                                aggp[:],
                                g[:].rearrange("p (j d) -> p j d", d=H)[:, j, :],
                                mk[:, (ci % 4) * P:(ci % 4 + 1) * P],
                                start=(ci == 0), stop=(ci == NCB - 1))
                            ci += 1
                    aggs = work.tile([H, P], F32, tag="aggs")
                    nc.vector.tensor_copy(aggs[:], aggp[:])
                    outp = psum.tile([P, 2 * H], F32, tag="outp")
                    nc.tensor.matmul(outp[:, :fout], aggs[:], Wl_t[:],
                                     start=True, stop=True)
                    o1 = work.tile([P, 2 * H], F32, tag="o1")
                    nc.vector.tensor_scalar_mul(o1[:, :fout], outp[:, :fout],
                                                dinvp_t[:, b:b + 1])
                    nc.vector.tensor_tensor(o1[:, :fout], o1[:, :fout],
                                            bl_t[:, :fout],
                                            op=mybir.AluOpType.add)
                    yield b, o1

        # ================= conv1 =================
        for b, o1 in conv(idx1, cslot1, NCBQ1, NCB1, xs1_rows, N,
                          W1_t, b1_t, H, True):
            xs2t = work.tile([P, H], F32, tag="xs2t")
            nc.scalar.activation(xs2t[:], o1[:, :H],
                                 mybir.ActivationFunctionType.Relu,
                                 scale=dinvp_t[:, b:b + 1])
            nc.sync.dma_start(cc_in[b * P:(b + 1) * P, :], xs2t[:])

        # ================= AllGather =================
        nc.gpsimd.collective_compute(
            "AllGather", mybir.AluOpType.bypass,
            replica_groups=[list(range(NCORES))],
            ins=[cc_in[:]], outs=[cc_out[:]])

        # ================= embedding MLP (independent) =================
        smT = []
        for k in range(D_EMB // P):
            t = consts.tile([P, G_pad], F32, tag=f"smT{k}")
            nc.sync.dma_start(t[:], smilesT[k * P:(k + 1) * P, :])
            smT.append(t)
        NS = [(0, 512), (512, G_pad - 512)] if G_pad > 512 else [(0, G_pad)]
        e1T = []
        for m in range(1024 // P):
            e1 = consts.tile([P, G_pad], F32, tag=f"e1T{m}")
            for (n0, nw) in NS:
                pm = psum1.tile([P, 512], F32, tag="mlpA")
                for k in range(D_EMB // P):
                    nc.tensor.matmul(
                        pm[:, :nw],
                        embW1_t[k][:, m * P:(m + 1) * P],
                        smT[k][:, n0:n0 + nw],
                        start=(k == 0), stop=(k == D_EMB // P - 1))
                nc.scalar.activation(e1[:, n0:n0 + nw], pm[:, :nw],
                                     mybir.ActivationFunctionType.Relu,
                                     bias=embb1_t[:, m:m + 1])
            e1T.append(e1)
        e2T = consts.tile([H, G_pad], F32, tag="e2T")
        for (n0, nw) in NS:
            pm = psum1.tile([P, 512], F32, tag="mlpA")
            for m in range(1024 // P):
                nc.tensor.matmul(pm[:H, :nw], embW2_t[m][:],
                                 e1T[m][:, n0:n0 + nw],
                                 start=(m == 0), stop=(m == 1024 // P - 1))
            nc.scalar.activation(e2T[:, n0:n0 + nw], pm[:H, :nw],
                                 mybir.ActivationFunctionType.Identity,
                                 bias=embb2_t[:])
        # Copy needs float bias; use Identity-style add via vector instead:
        # (handled above with bias AP; if Copy rejects AP bias, fallback below)

        # ================= conv2 + pooling =================
        poolA = psum1.tile([P, 512], F32, tag="poolA")
        if G_pad > 512:
            poolB = psum1.tile([P, G_pad - 512], F32, tag="poolB")
        slab_rows = cc_out[:]
        for b, o2 in conv(idx2, cslot2, NCBQ2, NCB2, slab_rows, SLAB,
                          W2_t, b2_t, 2 * H, False):
            gm = mpool.tile([P, G_pad], F32, tag="gmask")
            nc.vector.tensor_tensor(
                gm[:], _bc(batchl_t[:, b:b + 1], [G_pad]),
                giota_t[:],
                op=mybir.AluOpType.is_equal)
            nc.tensor.matmul(poolA[:], o2[:, :2 * H], gm[:, :512],
                             start=(b == 0), stop=(b == NBLK - 1))
            if G_pad > 512:
                nc.tensor.matmul(poolB[:], o2[:, :2 * H], gm[:, 512:],
                                 start=(b == 0), stop=(b == NBLK - 1))

        # pooled mean -> gfc -> fc1 -> fcf
        poolm = consts.tile([P, G_pad], F32, tag="poolm")
        nc.vector.tensor_tensor(poolm[:, :512], poolA[:],
                                cntinv_t[:, :512],
                                op=mybir.AluOpType.mult)
        if G_pad > 512:
            nc.vector.tensor_tensor(
                poolm[:, 512:], poolB[:],
                cntinv_t[:, 512:],
                op=mybir.AluOpType.mult)
        gfcT = consts.tile([H, G_pad], F32, tag="gfcT")
        for (n0, nw) in NS:
            pm = psum1.tile([P, 512], F32, tag="mlpB")
            nc.tensor.matmul(pm[:H, :nw], gfcW_t[:], poolm[:, n0:n0 + nw],
                             start=True, stop=True)
            nc.scalar.activation(gfcT[:, n0:n0 + nw], pm[:H, :nw],
                                 mybir.ActivationFunctionType.Identity,
                                 bias=gfcb_t[:])
        c1T = consts.tile([H, G_pad], F32, tag="c1T")
        for (n0, nw) in NS:
            pm = psum1.tile([P, 512], F32, tag="mlpA")
            nc.tensor.matmul(pm[:H, :nw], fc1Wa_t[:], e2T[:, n0:n0 + nw],
                             start=True, stop=False)
            nc.tensor.matmul(pm[:H, :nw], fc1Wb_t[:], gfcT[:, n0:n0 + nw],
                             start=False, stop=True)
            nc.scalar.activation(c1T[:, n0:n0 + nw], pm[:H, :nw],
                                 mybir.ActivationFunctionType.Identity,
                                 bias=fc1b_t[:])
        outT = consts.tile([1, G_pad], F32, tag="outT")
        for (n0, nw) in NS:
            pm = psum1.tile([P, 512], F32, tag="mlpB")
            nc.tensor.matmul(pm[:1, :nw], fcfW_t[:], c1T[:, n0:n0 + nw],
                             start=True, stop=True)
            nc.scalar.activation(outT[:, n0:n0 + nw], pm[:1, :nw],
                                 mybir.ActivationFunctionType.Identity,
                                 bias=fcfb_t[:])
        nc.sync.dma_start(out_d[:], outT[:])

    nc.compile()
    return nc


# ---------------- runner ----------------------------------------------------
class _Runner:
    def __init__(self, nc, n_cores):
        install_neuronx_cc_hook()
        self.nc = nc
        self.n_cores = n_cores
        in_names, out_names, out_avals, zero_outs = [], [], [], []
        pname = nc.partition_id_tensor.name if nc.partition_id_tensor else None
        for alloc in nc.m.functions[0].allocations:
            if not isinstance(alloc, mybir.MemoryLocationSet):
                continue
            name = alloc.memorylocations[0].name
            if alloc.kind == "ExternalInput":
                if name != pname:
                    in_names.append(name)
            elif alloc.kind == "ExternalOutput":
                shape = tuple(alloc.tensor_shape)
                dtype = mybir.dt.np(alloc.dtype)
                out_names.append(name)
                out_avals.append(jax.core.ShapedArray(shape, dtype))
                zero_outs.append(np.zeros(shape, dtype))
        self.in_names, self.out_names = in_names, out_names
        self.zero_outs = zero_outs
        n_params, n_outs = len(in_names), len(out_names)
        all_in = list(in_names) + out_names
        if pname is not None:
            all_in.append(pname)

        def _body(*args):
            operands = list(args)
            if pname is not None:
                operands.append(partition_id_tensor())
            outs = _bass_exec_p.bind(
                *operands, out_avals=tuple(out_avals), in_names=tuple(all_in),
                out_names=tuple(out_names), lowering_input_output_aliases=(),
                sim_require_finite=False, sim_require_nnan=False, nc=nc)
            return tuple(outs)

        donate = tuple(range(n_params, n_params + n_outs))
        devices = jax.devices()[:n_cores]
        self.mesh = Mesh(np.asarray(devices), ("core",))
        in_specs = (PartitionSpec("core"),) * (n_params + n_outs)
        out_specs = (PartitionSpec("core"),) * n_outs
        self.fn = jax.jit(
            shard_map(_body, mesh=self.mesh, in_specs=in_specs,
                      out_specs=out_specs, check_rep=False),
            donate_argnums=donate, keep_unused=True)

    def run(self, in_maps, n_iters=1):
        per_core = [[np.ascontiguousarray(m[n]) for n in self.in_names]
                    for m in in_maps]
        sh = NamedSharding(self.mesh, PartitionSpec("core"))
        dev = [jax.device_put(
            np.concatenate([per_core[c][i] for c in range(self.n_cores)], 0), sh)
            for i in range(len(self.in_names))]
        jax.block_until_ready(dev)
        times, outs = [], None
        for _ in range(n_iters):
            zouts = [np.concatenate([z] * self.n_cores, 0)
                     for z in self.zero_outs]
            t0 = time.perf_counter()
            outs = self.fn(*dev, *zouts)
            jax.block_until_ready(outs)
            times.append(time.perf_counter() - t0)
        res = []
        for c in range(self.n_cores):
            d = {}
            for i, nm in enumerate(self.out_names):
                a = np.asarray(outs[i])
                s0 = self.zero_outs[i].shape[0]
                d[nm] = a[c * s0:(c + 1) * s0]
            res.append(d)
        return res, times


_CACHE = {}


def _prepare(inputs):
    edge_index = np.asarray(inputs["edge_index"])
    batch = np.asarray(inputs["batch"])
    meta = _preprocess(edge_index, batch)
    NBLK = meta["NBLK"]
    V_pad = NBLK * P

    # conv1 table key: p-major position of node r
    def key1(r):
        return (r % P) * (N // P) + (r // P)

    ch1, NCBQ1, NCB1 = _build_chunks(meta, key1, (N + WIN - 1) // WIN)

    # conv2 table key: slab position owner*V_pad + blk*128 + slot
    own = np.empty(N, dtype=np.int64)
    pos = np.empty(N, dtype=np.int64)
    for ic, co in enumerate(meta["cores"]):
        n0, n1 = co["n0"], co["n1"]
        own[n0:n1] = ic
        pos[n0:n1] = co["blk_of"] * P + co["slot_of"]

    def key2(r):
        return own[r] * V_pad + pos[r]

    nwin2 = (NCORES * V_pad + WIN - 1) // WIN
    ch2, NCBQ2, NCB2 = _build_chunks(meta, key2, nwin2)

    Gmax = max(co["Gc"] for co in meta["cores"])
    G_pad = max(544, ((Gmax + 31) // 32) * 32)

    cfg = dict(NBLK=NBLK, NCBQ1=tuple(int(v) for v in NCBQ1), NCB1=NCB1,
               NCBQ2=tuple(int(v) for v in NCBQ2), NCB2=NCB2, G_pad=G_pad)

    # ---- shared (replicated) arrays ----
    x = np.asarray(inputs["x"], np.float32)
    x_pm = np.ascontiguousarray(
        x.reshape(N // P, P, H).transpose(1, 0, 2)).reshape(P, -1).astype(np.float16)
    deg = meta["deg"].astype(np.float32)
    deg_pm = np.ascontiguousarray(deg.reshape(N // P, P).T)
    iota128 = np.tile(np.arange(P, dtype=np.float32).reshape(1, P), (P, 1))
    giota = np.tile(np.arange(G_pad, dtype=np.float32).reshape(1, G_pad), (P, 1))
    smiles = np.asarray(inputs["smiles_embedding"], np.float32)[:, 0, :]  # [B,768]

    shared = dict(
        x_pm=x_pm, deg_pm=deg_pm, iota128=iota128, giota=giota,
        emb_W1=np.asarray(inputs["emb_W1"], np.float32),
        emb_b1=np.asarray(inputs["emb_b1"], np.float32).reshape(-1, 1),
        emb_W2=np.asarray(inputs["emb_W2"], np.float32),
        emb_b2=np.asarray(inputs["emb_b2"], np.float32).reshape(-1, 1),
        conv1_W=np.asarray(inputs["conv1_W"], np.float32),
        conv1_b=np.tile(np.asarray(inputs["conv1_b"], np.float32).reshape(1, -1), (P, 1)),
        conv2_W=np.asarray(inputs["conv2_W"], np.float32),
        conv2_b=np.tile(np.asarray(inputs["conv2_b"], np.float32).reshape(1, -1), (P, 1)),
        gcn_fc_W=np.asarray(inputs["gcn_fc_W"], np.float32),
        gcn_fc_b=np.asarray(inputs["gcn_fc_b"], np.float32).reshape(-1, 1),
        fc1_W=np.asarray(inputs["fc1_W"], np.float32),
        fc1_b=np.asarray(inputs["fc1_b"], np.float32).reshape(-1, 1),
        fcf_W=np.asarray(inputs["fcf_W"], np.float32),
        fcf_b=np.asarray(inputs["fcf_b"], np.float32).reshape(1, 1),
    )

    in_maps = []
    for ic, co in enumerate(meta["cores"]):
        perm = co["perm"]
        valid = perm >= 0
        pm = np.clip(perm, 0, None)
        dp = np.where(valid, deg[co["n0"]:co["n1"]][pm], 1.0).astype(np.float32)
        bl = np.where(valid,
                      (batch[co["n0"]:co["n1"]].astype(np.int64)[pm] - co["g0"]),
                      10 ** 6).astype(np.float32)
        cnt = np.zeros(G_pad, np.float32)
        gc = np.bincount(batch[co["n0"]:co["n1"]] - co["g0"],
                         minlength=co["Gc"]).astype(np.float32)
        cnt[:co["Gc"]] = gc
        smT = np.zeros((D_EMB, G_pad), np.float32)
        smT[:, :co["Gc"]] = smiles[co["g0"]:co["g1"]].T
        m = dict(shared)
        m.update(
            deg_perm=np.ascontiguousarray(dp.reshape(NBLK, P).T),
            batchl=np.ascontiguousarray(bl.reshape(NBLK, P).T),
            cslot1=np.ascontiguousarray(
                ch1[ic]["cslot"].reshape(NBLK * NCB1, P).T).astype(np.float32),
            cslot2=np.ascontiguousarray(
                ch2[ic]["cslot"].reshape(NBLK * NCB2, P).T).astype(np.float32),
            idx1=_pack_idx(ch1[ic]["ckey"], NCBQ1),
            idx2=_pack_idx(ch2[ic]["ckey"], NCBQ2),
            smilesT=smT, cntg=np.tile(cnt.reshape(1, -1), (P, 1)),
        )
        in_maps.append(m)
    return cfg, meta, in_maps


def kernel(**inputs):
    cfg, meta, in_maps = _prepare(inputs)
    key = tuple(sorted(cfg.items()))
    if key not in _CACHE:
        nc = build_kernel(cfg)
        _CACHE[key] = _Runner(nc, NCORES)
    runner = _CACHE[key]
    res, _ = runner.run(in_maps)
    out = np.zeros((B, 1), np.float32)
    for c, co in enumerate(meta["cores"]):
        out[co["g0"]:co["g1"], 0] = res[c]["out"][0, :co["Gc"]]
    return out


if __name__ == "__main__":
    d = np.load("/root/problem/ref_cache.npz")
    inputs = {k: d[k] for k in d.files if k != "expected"}
    exp = d["expected"]
    got = kernel(**inputs)
    err = np.abs(got - exp).max() / (np.abs(exp).max() + 1e-12)
    print(f"Relative error: {err:.3e}")
